# revision 23
# baseline (speedup 1.0000x reference)
"""Trainium2 Bass kernel for k-reciprocal GIN graph network (retrieval_knn).

Pipeline per core (row-shard of N across 8 cores, full inputs on every core):
  0a. normalize local query rows, transpose -> xqnT (SBUF-resident stationary)
  0b. normalize all rows, transpose -> xnT tiles in DRAM (moving operand)
  1.  sim = xqn @ xn.T strip-by-strip on PE (fp32), per-tile top-8 candidates
      via DVE max8/max_index, merged to per-row top-8 + global indices.
  1.5 all-gather the per-row top-6 index table across cores.
  2.  neighbor aggregation: gather top-6 x rows via indirect DMA, reciprocity
      check i in top6(j) by index membership, weighted sum -> aggr;
      h = 1.3*x + aggr -> hT in DRAM (transposed).
  3.  MLP (w1/relu/w2) in transposed layout, BN stats via all-reduce,
      classifier GEMM -> logitsT output per core.
"""
import numpy as np

import concourse.bass as bass
import concourse.mybir as mybir
import concourse.tile as tile
from concourse import bacc, bass_utils
from concourse.masks import make_identity

P = 128
F32 = mybir.dt.float32
I32 = mybir.dt.int32
U32 = mybir.dt.uint32
AF = mybir.ActivationFunctionType
ALU = mybir.AluOpType

GIN_EPS = 0.3
BN_EPS = 1e-5


def build_kernel(N=8192, D=2048, NCORES=8, CPAD=768, K_SEL=6, debug=False,
                 stop_stage=99, mlp_f32r=True, dist_f32r=True, max8_psum=True,
                 fake_collectives=False):
    NL = N // NCORES          # local rows per core
    KT = D // P               # contraction tiles
    MT = NL // P              # local row strips
    NSB = 512                 # n-superblock width
    NB = N // NSB             # n superblocks
    OT = D // P               # output-feature tiles for MLP
    CT = CPAD // P            # class tiles
    M_GRP = min(4, MT)        # strips per phase-1 psum group
    N_GRP = min(4, OT)        # ot per mlp psum group
    C_GRP = min(4, CT)
    JG = NSB // P             # x row-tiles per xnT tile
    JSTG = 2                  # row-tiles per staging buffer

    nc = bacc.Bacc("TRN2", target_bir_lowering=False, debug=False,
                   num_devices=NCORES)
    DSDT = F32R if dist_f32r else F32     # dist operand storage dtype

    MMDT = mybir.dt.float32r if mlp_f32r else F32   # mlp storage dtype
    xf = nc.dram_tensor("xf", [N, D], F32, kind="ExternalInput")
    xq = nc.dram_tensor("xq", [NL, D], F32, kind="ExternalInput")
    rowid = nc.dram_tensor("rowid", [NL, 1], F32, kind="ExternalInput")
    w1t = nc.dram_tensor("w1t", [D, D], MMDT, kind="ExternalInput")
    w2t = nc.dram_tensor("w2t", [D, D], MMDT, kind="ExternalInput")
    wct = nc.dram_tensor("wct", [D, CPAD], MMDT, kind="ExternalInput")
    b1r = nc.dram_tensor("b1r", [P, OT], F32, kind="ExternalInput")
    b2r = nc.dram_tensor("b2r", [P, OT], F32, kind="ExternalInput")
    gar = nc.dram_tensor("gar", [P, OT], F32, kind="ExternalInput")
    ber = nc.dram_tensor("ber", [P, OT], F32, kind="ExternalInput")

    logitsT = nc.dram_tensor("logitsT", [CPAD, NL], F32, kind="ExternalOutput")
    if debug:
        idx_dbg = nc.dram_tensor("idx_dbg", [NL, 8], F32, kind="ExternalOutput")
        agg_dbg = nc.dram_tensor("agg_dbg", [NL, D], F32, kind="ExternalOutput")
        wk_dbg = nc.dram_tensor("wk_dbg", [P, K_SEL], F32, kind="ExternalOutput")

    def normalize_tile(nc, sb_pool, x_sb):
        """x_sb [128, D] -> xn_sb [128, D] (L2-normalized rows)."""
        sq = sb_pool.tile([P, D], F32, tag="nrm_sq", bufs=1)
        ssq = sb_pool.tile([P, 1], F32, tag="nrm_ss")
        nrm = sb_pool.tile([P, 1], F32, tag="nrm_n")
        rinv = sb_pool.tile([P, 1], F32, tag="nrm_r")
        xn_sb = sb_pool.tile([P, D], F32, tag="nrm_out")
        nc.scalar.activation(sq[:], x_sb[:], AF.Square, accum_out=ssq[:])
        nc.scalar.activation(nrm[:], ssq[:], AF.Sqrt)
        nc.vector.reciprocal(rinv[:], nrm[:])
        nc.vector.tensor_scalar_mul(xn_sb[:], x_sb[:], rinv[:, :1])
        return xn_sb, rinv

    with tile.TileContext(nc) as tc:
        with (
            tc.tile_pool(name="const", bufs=1) as const_pool,
            tc.tile_pool(name="dram", bufs=1, space="DRAM") as dram,
            tc.tile_pool(name="keep", bufs=1) as keep,
        ):
            ident = const_pool.tile([P, P], F32)
            make_identity(nc, ident[:])

            xnT = [dram.tile([D, NSB], DSDT, tag=f"xnT{g}", name=f"xnT{g}")
                   for g in range(NB)]
            rinv_tbl = dram.tile([N, 1], F32)
            w1_src, w2_src, wc_src = w1t, w2t, wct
            idx_loc = dram.tile([NL, K_SEL], F32)
            idx_full = dram.tile([N, K_SEL], F32)
            stats_loc = dram.tile([P, 2 * OT], F32)
            stats_glob = dram.tile([P, 2 * OT], F32)

            top8s = [keep.tile([P, 8], F32, tag=f"top8_{m}", name=f"top8_{m}")
                     for m in range(MT)]
            idx6s = [keep.tile([P, K_SEL], I32, tag=f"idx6_{m}", name=f"idx6_{m}")
                     for m in range(MT)]
            piota_i = const_pool.tile([P, 1], I32)
            nc.gpsimd.iota(piota_i[:], [[0, 1]], base=0, channel_multiplier=NB * 8)
            piota = const_pool.tile([P, 1], F32)
            nc.vector.tensor_copy(piota[:], piota_i[:])
            piota8_i = const_pool.tile([P, 1], I32)
            nc.gpsimd.iota(piota8_i[:], [[0, 1]], base=0, channel_multiplier=8)
            piota8 = const_pool.tile([P, 1], F32)
            nc.vector.tensor_copy(piota8[:], piota8_i[:])

            with tc.tile_pool(name="trps", bufs=4, space="PSUM") as trps:
                # ======== phases 0a/0b/1 (xqnT + p0 SBUF scoped here) ========
                with (
                    tc.tile_pool(name="p0", bufs=2) as p0,
                    tc.tile_pool(name="xqn", bufs=1) as xqn_pool,
                ):
                    xqnT = xqn_pool.tile([P, KT * NL], DSDT)  # kt-major blocks
                    for m in range(MT):
                        x_sb = p0.tile([P, D], F32, tag="ld")
                        nc.sync.dma_start(x_sb[:], xq[m * P:(m + 1) * P, :])
                        xn_sb, _ = normalize_tile(nc, p0, x_sb)
                        for kt in range(KT):
                            ps = trps.tile([P, P], F32, tag="tr")
                            nc.tensor.transpose(
                                ps[:], xn_sb[:, kt * P:(kt + 1) * P], ident[:])
                            nc.scalar.copy(
                                xqnT[:, kt * NL + m * P: kt * NL + (m + 1) * P], ps[:])

                    # ---- phase 0b
                    for g in range(NB):
                        for js in range(JG // JSTG):
                            stage = p0.tile([P, KT * JSTG * P], DSDT, tag="stf")
                            for j2 in range(JSTG):
                                j4 = js * JSTG + j2
                                j = g * JG + j4
                                x_sb = p0.tile([P, D], F32, tag="ld")
                                nc.sync.dma_start(x_sb[:], xf[j * P:(j + 1) * P, :])
                                xn_sb, rinv_sb = normalize_tile(nc, p0, x_sb)
                                nc.sync.dma_start(
                                    rinv_tbl[j * P:(j + 1) * P, :], rinv_sb[:])
                                W = JSTG * P
                                for kt in range(KT):
                                    ps = trps.tile([P, P], F32, tag="tr")
                                    nc.tensor.transpose(
                                        ps[:], xn_sb[:, kt * P:(kt + 1) * P], ident[:])
                                    nc.scalar.copy(
                                        stage[:, kt * W + j2 * P: kt * W + (j2 + 1) * P],
                                        ps[:])
                            dst = xnT[g][:].rearrange("(kt p) n -> p kt n", p=P)[
                                :, :, js * JSTG * P:(js + 1) * JSTG * P]
                            nc.sync.dma_start(
                                dst, stage[:].rearrange("p (kt c) -> p kt c", kt=KT))

                    # ---- phase 1
                    with (
                        tc.tile_pool(name="p1", bufs=3) as p1,
                        tc.tile_pool(name="p1c", bufs=2) as p1c,
                        tc.tile_pool(name="p1ps", bufs=1, space="PSUM") as p1ps,
                    ):
                        n_grp = (MT + M_GRP - 1) // M_GRP
                        for grp in range(n_grp):
                            ms = [grp * M_GRP + i for i in range(M_GRP)
                                  if grp * M_GRP + i < MT]
                            cvs = {m: p1c.tile([P, NB * 8], F32, tag=f"cv{m % M_GRP}",
                                               name=f"cv_{m}") for m in ms}
                            cgs = {m: p1c.tile([P, NB * 8], F32, tag=f"cg{m % M_GRP}",
                                               name=f"cg_{m}") for m in ms}
                            for n in range(NB):
                                psums = {m: p1ps.tile([P, NSB], F32,
                                                      tag=f"mm{m % M_GRP}",
                                                      name=f"ps_{m}") for m in ms}
                                for kt in range(KT):
                                    slab = p1.tile([P, NSB], DSDT, tag="slab")
                                    nc.sync.dma_start(
                                        slab[:], xnT[n][kt * P:(kt + 1) * P, :])
                                    for m in ms:
                                        nc.tensor.matmul(
                                            psums[m][:],
                                            lhsT=xqnT[:, kt * NL + m * P:
                                                      kt * NL + (m + 1) * P],
                                            rhs=slab[:],
                                            start=(kt == 0), stop=(kt == KT - 1))
                                for m in ms:
                                    if max8_psum:
                                        sim_sb = psums[m]
                                    else:
                                        sim_sb = p1.tile([P, NSB], F32, tag="simc")
                                        nc.scalar.copy(sim_sb[:], psums[m][:])
                                    cv8 = cvs[m][:, n * 8:(n + 1) * 8]
                                    nc.vector.max(cv8, sim_sb[:])
                                    ci_u = p1.tile([P, 8], U32, tag="ciu")
                                    nc.vector.max_index(ci_u[:], cv8, sim_sb[:])
                                    cg8 = cgs[m][:, n * 8:(n + 1) * 8]
                                    nc.vector.tensor_copy(cg8, ci_u[:])
                                    if n > 0:
                                        nc.vector.tensor_scalar_add(
                                            cg8, cg8, float(n * NSB))
                            # merge per strip: approx top-8 + their global indices
                            for m in ms:
                                top8a = p1.tile([P, 8], F32, tag="top8a")
                                nc.vector.max(top8a[:], cvs[m][:])
                                pos_u = p1.tile([P, 8], U32, tag="posu")
                                nc.vector.max_index(pos_u[:], top8a[:], cvs[m][:])
                                pos_f = p1.tile([P, 8], F32, tag="posf")
                                nc.vector.tensor_copy(pos_f[:], pos_u[:])
                                nc.vector.tensor_scalar_add(
                                    pos_f[:], pos_f[:], piota[:, :1])
                                abs_i = p1.tile([P, 8], I32, tag="absi")
                                nc.vector.tensor_copy(abs_i[:], pos_f[:])
                                gsc = dram.tile([P * NB * 8, 1], F32, tag="gsc",
                                                bufs=4, name=f"gsc_{m}")
                                nc.sync.dma_start(
                                    gsc[:].rearrange("(p c) one -> p (c one)", p=P),
                                    cgs[m][:])
                                gidx8 = p1.tile([P, 8], F32, tag="gfx")
                                for k in range(8):
                                    nc.gpsimd.indirect_dma_start(
                                        out=gidx8[:, k:k + 1], out_offset=None,
                                        in_=gsc[:, :],
                                        in_offset=bass.IndirectOffsetOnAxis(
                                            ap=abs_i[:, k:k + 1], axis=0))
                                if not dist_f32r:
                                    nc.vector.tensor_copy(top8s[m][:], top8a[:])
                                    nc.vector.tensor_copy(idx6s[m][:],
                                                          gidx8[:, :K_SEL])
                                    nc.sync.dma_start(
                                        idx_loc[m * P:(m + 1) * P, :],
                                        gidx8[:, :K_SEL])
                                    if debug and m == 0:
                                        nc.sync.dma_start(idx_dbg[0:P, 0:K_SEL],
                                                          gidx8[:, :K_SEL])
                                    continue
                                # ---- exact refinement of the 8 candidates ----
                                idx8 = p1.tile([P, 8], I32, tag="idx8")
                                nc.vector.tensor_copy(idx8[:], gidx8[:])
                                xq_sb = p0.tile([P, D], F32, tag="ld")
                                nc.sync.dma_start(xq_sb[:],
                                                  xq[m * P:(m + 1) * P, :])
                                xqn_sb, _ = normalize_tile(nc, p0, xq_sb)
                                ex = p1.tile([P, 8], F32, tag="ex")
                                for k in range(8):
                                    xrow = p1.tile([P, D], F32, tag="rxrow")
                                    nc.gpsimd.indirect_dma_start(
                                        out=xrow[:], out_offset=None, in_=xf[:, :],
                                        in_offset=bass.IndirectOffsetOnAxis(
                                            ap=idx8[:, k:k + 1], axis=0))
                                    rig = p1.tile([P, 1], F32, tag="rig")
                                    nc.gpsimd.indirect_dma_start(
                                        out=rig[:], out_offset=None,
                                        in_=rinv_tbl[:, :],
                                        in_offset=bass.IndirectOffsetOnAxis(
                                            ap=idx8[:, k:k + 1], axis=0))
                                    prod = p1.tile([P, D], F32, tag="prod")
                                    nc.vector.tensor_tensor(
                                        prod[:], xqn_sb[:], xrow[:], op=ALU.mult)
                                    seg = p1.tile([P, KT], F32, tag="seg")
                                    nc.vector.tensor_reduce(
                                        out=seg[:],
                                        in_=prod[:].rearrange(
                                            "p (kt c) -> p kt c", kt=KT),
                                        op=ALU.add, axis=mybir.AxisListType.X)
                                    raw = p1.tile([P, 1], F32, tag="raw")
                                    nc.vector.tensor_reduce(
                                        out=raw[:], in_=seg[:], op=ALU.add,
                                        axis=mybir.AxisListType.X)
                                    nc.vector.tensor_tensor(
                                        ex[:, k:k + 1], raw[:], rig[:], op=ALU.mult)
                                # exact top-8 (sorted) + final index resolution
                                nc.vector.max(top8s[m][:], ex[:])
                                pos2_u = p1.tile([P, 8], U32, tag="pos2u")
                                nc.vector.max_index(pos2_u[:], top8s[m][:], ex[:])
                                pos2_f = p1.tile([P, 8], F32, tag="pos2f")
                                nc.vector.tensor_copy(pos2_f[:], pos2_u[:])
                                nc.vector.tensor_scalar_add(
                                    pos2_f[:], pos2_f[:], piota8[:, :1])
                                abs2 = p1.tile([P, 8], I32, tag="abs2")
                                nc.vector.tensor_copy(abs2[:], pos2_f[:])
                                gsc2 = dram.tile([P * 8, 1], F32, tag="gsc2",
                                                 bufs=4, name=f"gsc2_{m}")
                                nc.sync.dma_start(
                                    gsc2[:].rearrange("(p c) one -> p (c one)", p=P),
                                    gidx8[:])
                                fidx = p1.tile([P, K_SEL], F32, tag="fidx")
                                for k in range(K_SEL):
                                    nc.gpsimd.indirect_dma_start(
                                        out=fidx[:, k:k + 1], out_offset=None,
                                        in_=gsc2[:, :],
                                        in_offset=bass.IndirectOffsetOnAxis(
                                            ap=abs2[:, k:k + 1], axis=0))
                                nc.vector.tensor_copy(idx6s[m][:], fidx[:])
                                nc.sync.dma_start(
                                    idx_loc[m * P:(m + 1) * P, :], fidx[:])
                                if debug and m == 0:
                                    nc.sync.dma_start(idx_dbg[0:P, 0:K_SEL], fidx[:])

                # ======== phase 1.5: all-gather index table ========
                if stop_stage >= 2:
                    if NCORES == 1 or fake_collectives:
                        nc.gpsimd.dma_start(idx_full[:NL, :], idx_loc[:, :])
                    else:
                        nc.gpsimd.collective_compute(
                            "AllGather", ALU.bypass,
                            replica_groups=[list(range(NCORES))],
                            ins=[idx_loc.opt()], outs=[idx_full.opt()])

                # ======== phase 2: gather neighbors, aggregate, h -> hT ========
                with (
                    tc.tile_pool(name="p2", bufs=3) as p2,
                    tc.tile_pool(name="p2b", bufs=2) as p2b,
                ):
                    for m in range(MT if stop_stage >= 3 else 0):
                        rid = p2.tile([P, 1], F32, tag="rid")
                        nc.sync.dma_start(rid[:], rowid[m * P:(m + 1) * P, :])
                        aggr = p2b.tile([P, D], F32, tag="aggr")
                        for k in range(K_SEL):
                            xrow = p2.tile([P, D], F32, tag="xrow")
                            nc.gpsimd.indirect_dma_start(
                                out=xrow[:], out_offset=None, in_=xf[:, :],
                                in_offset=bass.IndirectOffsetOnAxis(
                                    ap=idx6s[m][:, k:k + 1], axis=0))
                            nbi = p2.tile([P, K_SEL], F32, tag="nbi")
                            nc.gpsimd.indirect_dma_start(
                                out=nbi[:], out_offset=None, in_=idx_full[:, :],
                                in_offset=bass.IndirectOffsetOnAxis(
                                    ap=idx6s[m][:, k:k + 1], axis=0))
                            eqm = p2.tile([P, K_SEL], F32, tag="eqm")
                            nc.vector.tensor_scalar(
                                eqm[:], nbi[:], rid[:, :1], None, op0=ALU.is_equal)
                            wk = p2.tile([P, 1], F32, tag="wk")
                            nc.vector.tensor_reduce(
                                out=wk[:], in_=eqm[:], op=ALU.max,
                                axis=mybir.AxisListType.X)
                            if k == 0:
                                nc.vector.tensor_scalar_mul(aggr[:], xrow[:], wk[:, :1])
                            else:
                                wx = p2.tile([P, D], F32, tag="wx")
                                nc.vector.tensor_scalar_mul(wx[:], xrow[:], wk[:, :1])
                                nc.vector.tensor_add(aggr[:], aggr[:], wx[:])
                            if debug and m == 0:
                                nc.sync.dma_start(wk_dbg[:, k:k + 1], wk[:])
                        if debug:
                            nc.sync.dma_start(agg_dbg[m * P:(m + 1) * P, :], aggr[:])
                        xq_sb = p2.tile([P, D], F32, tag="xq2")
                        nc.sync.dma_start(xq_sb[:], xq[m * P:(m + 1) * P, :])
                        h_sb = p2b.tile([P, D], F32, tag="hsb")
                        nc.vector.tensor_scalar(
                            h_sb[:], xq_sb[:], float(1.0 + GIN_EPS), None, op0=ALU.mult)
                        nc.vector.tensor_add(h_sb[:], h_sb[:], aggr[:])
                        stage = p2b.tile([P, KT * P], MMDT, tag="sth")
                        for kt in range(KT):
                            ps = trps.tile([P, P], F32, tag="tr")
                            nc.tensor.transpose(
                                ps[:], h_sb[:, kt * P:(kt + 1) * P], ident[:])
                            nc.scalar.copy(stage[:, kt * P:(kt + 1) * P], ps[:])
                        dst = hT[:].rearrange("(kt p) i -> p kt i", p=P)[
                            :, :, m * P:(m + 1) * P]
                        nc.sync.dma_start(
                            dst, stage[:].rearrange("p (kt c) -> p kt c", kt=KT))

            # ======== phase 3: MLP + BN + classifier ========
            if stop_stage >= 4:
                with (
                    tc.tile_pool(name="p3", bufs=3) as p3,
                    tc.tile_pool(name="p3w", bufs=3) as p3w,
                    tc.tile_pool(name="p3s", bufs=1) as p3s,
                    tc.tile_pool(name="p3ps", bufs=1, space="PSUM") as p3ps,
                ):
                    b1_sb = p3s.tile([P, OT], F32)
                    b2_sb = p3s.tile([P, OT], F32)
                    ga_sb = p3s.tile([P, OT], F32)
                    be_sb = p3s.tile([P, OT], F32)
                    nc.sync.dma_start(b1_sb[:], b1r[:, :])
                    nc.sync.dma_start(b2_sb[:], b2r[:, :])
                    nc.sync.dma_start(ga_sb[:], gar[:, :])
                    nc.sync.dma_start(be_sb[:], ber[:, :])

                    def mlp_layer(src, dst, wt, bias_sb, relu, stats):
                        src_dt = src.dtype if hasattr(src, "dtype") else MMDT
                        for og in range((OT + N_GRP - 1) // N_GRP):
                            ots = [og * N_GRP + i for i in range(N_GRP)
                                   if og * N_GRP + i < OT]
                            psums = {o: p3ps.tile([P, NL], F32, tag=f"mm{o % N_GRP}",
                                                  name=f"ps3_{o}") for o in ots}
                            for kt in range(KT):
                                hs = p3.tile([P, NL], MMDT, tag="hs")
                                nc.sync.dma_start(hs[:], src[kt * P:(kt + 1) * P, :])
                                for o in ots:
                                    w_sb = p3w.tile([P, P], MMDT, tag="w")
                                    nc.sync.dma_start(
                                        w_sb[:],
                                        wt[kt * P:(kt + 1) * P, o * P:(o + 1) * P])
                                    for ns in range(0, NL, NSB):
                                        nw = min(NSB, NL - ns)
                                        nc.tensor.matmul(
                                            psums[o][:, ns:ns + nw],
                                            lhsT=w_sb[:],
                                            rhs=hs[:, ns:ns + nw],
                                            start=(kt == 0), stop=(kt == KT - 1))
                            for o in ots:
                                if relu:
                                    o_sb = p3.tile([P, NL], MMDT, tag="osbr")
                                    nc.scalar.activation(
                                        o_sb[:], psums[o][:], AF.Relu,
                                        bias=bias_sb[:, o:o + 1])
                                else:
                                    o_sb = p3.tile([P, NL], F32, tag="osb")
                                    nc.scalar.activation(
                                        o_sb[:], psums[o][:], AF.Identity,
                                        bias=bias_sb[:, o:o + 1],
                                        accum_out=stats[0][:, o:o + 1])
                                    sq = p3.tile([P, NL], F32, tag="sq3")
                                    nc.scalar.activation(
                                        sq[:], o_sb[:], AF.Square,
                                        accum_out=stats[1][:, o:o + 1])
                                nc.sync.dma_start(dst[o * P:(o + 1) * P, :], o_sb[:])

                    mlp_layer(hT, h1T, w1_src, b1_sb, relu=True, stats=None)
                    sum_h = p3s.tile([P, OT], F32)
                    sum_h2 = p3s.tile([P, OT], F32)
                    mlp_layer(h1T, h2T, w2_src, b2_sb, relu=False, stats=(sum_h, sum_h2))

                    # BN stats all-reduce
                    st_sb = p3s.tile([P, 2 * OT], F32)
                    nc.vector.tensor_copy(st_sb[:, :OT], sum_h[:])
                    nc.vector.tensor_copy(st_sb[:, OT:], sum_h2[:])
                    nc.sync.dma_start(stats_loc[:, :], st_sb[:])
                    if NCORES == 1 or fake_collectives:
                        nc.gpsimd.dma_start(stats_glob[:, :], stats_loc[:, :])
                    else:
                        nc.gpsimd.collective_compute(
                            "AllReduce", ALU.add,
                            replica_groups=[list(range(NCORES))],
                            ins=[stats_loc.opt()], outs=[stats_glob.opt()])
                    stg = p3s.tile([P, 2 * OT], F32)
                    nc.sync.dma_start(stg[:], stats_glob[:, :])
                    mean = p3s.tile([P, OT], F32)
                    var = p3s.tile([P, OT], F32)
                    scale = p3s.tile([P, OT], F32)
                    shift = p3s.tile([P, OT], F32)
                    nc.vector.tensor_scalar_mul(mean[:], stg[:, :OT], 1.0 / N)
                    nc.vector.tensor_scalar_mul(var[:], stg[:, OT:], 1.0 / N)
                    msq = p3s.tile([P, OT], F32)
                    nc.vector.tensor_tensor(msq[:], mean[:], mean[:], op=ALU.mult)
                    nc.vector.tensor_sub(var[:], var[:], msq[:])
                    nc.vector.tensor_scalar_add(var[:], var[:], float(BN_EPS))
                    nc.scalar.activation(var[:], var[:], AF.Sqrt)
                    nc.vector.reciprocal(scale[:], var[:])   # rstd
                    nc.vector.tensor_tensor(scale[:], scale[:], ga_sb[:], op=ALU.mult)
                    nc.vector.tensor_tensor(shift[:], mean[:], scale[:], op=ALU.mult)
                    nc.vector.tensor_sub(shift[:], be_sb[:], shift[:])

                    # normalized activations resident in SBUF, then classifier
                    with tc.tile_pool(name="hn", bufs=1) as hn_pool:
                        hn = hn_pool.tile([P, KT * NL], MMDT)
                        for kt in range(KT):
                            hs = p3.tile([P, NL], F32, tag="hs")
                            nc.sync.dma_start(hs[:], h2T[kt * P:(kt + 1) * P, :])
                            nc.vector.tensor_scalar(
                                hn[:, kt * NL:(kt + 1) * NL], hs[:],
                                scale[:, kt:kt + 1], shift[:, kt:kt + 1],
                                op0=ALU.mult, op1=ALU.add)
                        for cg in range((CT + C_GRP - 1) // C_GRP):
                            cts = [cg * C_GRP + i for i in range(C_GRP)
                                   if cg * C_GRP + i < CT]
                            psums = {o: p3ps.tile([P, NL], F32, tag=f"mm{o % N_GRP}",
                                                  name=f"psc_{o}") for o in cts}
                            for kt in range(KT):
                                for o in cts:
                                    w_sb = p3w.tile([P, P], MMDT, tag="w")
                                    nc.sync.dma_start(
                                        w_sb[:],
                                        wc_src[kt * P:(kt + 1) * P,
                                               o * P:(o + 1) * P])
                                    for ns in range(0, NL, NSB):
                                        nw = min(NSB, NL - ns)
                                        nc.tensor.matmul(
                                            psums[o][:, ns:ns + nw],
                                            lhsT=w_sb[:],
                                            rhs=hn[:, kt * NL + ns:
                                                   kt * NL + ns + nw],
                                            start=(kt == 0), stop=(kt == KT - 1))
                            for o in cts:
                                o_sb = p3.tile([P, NL], F32, tag="osb")
                                nc.scalar.copy(o_sb[:], psums[o][:])
                                nc.sync.dma_start(
                                    logitsT[o * P:(o + 1) * P, :], o_sb[:])

    nc.compile()
    return nc


def _prep_inputs(x, w1, b1, w2, b2, gamma, beta, wc, NCORES=8, CPAD=768):
    N, D = x.shape
    NL = N // NCORES
    OT = D // P
    C = wc.shape[0]
    x = np.ascontiguousarray(x, np.float32)
    w1t = np.ascontiguousarray(np.asarray(w1, np.float32).T)
    w2t = np.ascontiguousarray(np.asarray(w2, np.float32).T)
    wct = np.zeros((D, CPAD), np.float32)
    wct[:, :C] = np.asarray(wc, np.float32).T

    def vec_r(v):
        return np.ascontiguousarray(np.asarray(v, np.float32).reshape(OT, P).T)

    base = {
        "xf": x, "w1t": w1t, "w2t": w2t, "wct": wct,
        "b1r": vec_r(b1), "b2r": vec_r(b2), "gar": vec_r(gamma), "ber": vec_r(beta),
    }
    in_maps = []
    for c in range(NCORES):
        m = dict(base)
        m["xq"] = np.ascontiguousarray(x[c * NL:(c + 1) * NL])
        m["rowid"] = np.arange(c * NL, (c + 1) * NL, dtype=np.float32).reshape(NL, 1)
        in_maps.append(m)
    return in_maps


_NC_CACHE = {}


def kernel(x, w1, b1, w2, b2, gamma, beta, wc):
    """Full-input entry point: returns [N, num_classes] float32 logits."""
    x = np.asarray(x)
    wc = np.asarray(wc)
    N, D = x.shape
    C = wc.shape[0]
    NCORES = 8
    CPAD = 768
    key = (N, D, NCORES, CPAD)
    if key not in _NC_CACHE:
        _NC_CACHE[key] = build_kernel(N=N, D=D, NCORES=NCORES, CPAD=CPAD)
    nc = _NC_CACHE[key]
    in_maps = _prep_inputs(x, w1, b1, w2, b2, gamma, beta, wc, NCORES, CPAD)
    res = bass_utils.run_bass_kernel_spmd(nc, in_maps, core_ids=list(range(NCORES)))
    out = np.concatenate(
        [res.results[c]["logitsT"].T[:, :C] for c in range(NCORES)], axis=0)
    return np.ascontiguousarray(out.astype(np.float32))
